# revision 2
# baseline (speedup 1.0000x reference)
"""CoAttention kernel for Trainium2, 8 NeuronCores, pure data parallel.

Math shortcut (exact, softmax shift-invariance): scores are additive in the
query index, so attention weights are query-independent:
    u[b] = softmax_j(tanh(text[b] @ Wt2) @ wa2[D:]) @ text[b]   (text out row)
    v[b] = softmax_r(tanh(img[b]  @ Wi1) @ wa1[D:]) @ img[b]    (img out row)
Wt1/bt1/Wi2/bi2/wa1[:D]/wa2[:D]/ba1/ba2 cancel exactly; host broadcasts over S.

Performance design (cost-model driven):
- ph1 (the big X@W matmuls) runs feature-major (W stationary, X^T moving) in
  fp8e4m3 DoubleRow: 0.5 cyc/col.  W pre-scaled x64 on host (fp8 subnormal
  dodge), 1/64 folded into tanh/exp activation scales.
- d = w.tanh(Y) is computed as matmul COLUMNS: lhsT = tanh tile (stationary,
  fp8 out of ACT), rhs = w column (fp8 DR) -> out [128 tok, 1].  Output free
  size 1 => near-zero engine time, and the scores land token-on-partition for
  the weighted sum directly: no d-row copies, no PE transposes.
- exp on ACT straight from the d-column PSUM tile, one call per superchunk,
  bf16 out (PE accepts mixed-dtype operands: bf16 lhsT x fp8 rhs).
- Weighted sums use fp8e3m4 token-major naturals (1.0 cyc/col, HALF the DMA
  bytes of bf16; e3m4 on N(0,1) has 1.3% RMS quant err -> ~4e-3 added absmax,
  inside the 2e-2 gate).  The 769th all-ones column produces the softmax
  denominator inside the same matmul; the final divide happens on HOST
  (outputs ship as numerator+denominator rows).
- img tokens ship PACKED (196/batch, no 256-pad): partial 68-row tiles are
  partition-sliced matmuls; junk PSUM rows are memset-zeroed once.
- Per-batch ups PSUM row is copied out split DVE(0:512)+Pool(512:769) so the
  single ups bank frees fast.  PSUM budget: ph1 2x[P,1024](4) + qcol 2 +
  ups 2 = 8 banks exactly.
"""

import numpy as np
import ml_dtypes

import concourse.bacc as bacc
import concourse.mybir as mybir
import concourse.tile as tile
from concourse.bass_utils import run_bass_kernel_spmd

B, S, R, D = 32, 512, 196, 768
NCORES = 8
BPC = B // NCORES          # 4 batches per core
P = 128
KO = 3                     # DoubleRow contraction groups of 256
NT = D // P                # 6 feature tiles
TTOK = BPC * S             # 2048 text tokens per core
ITOKP = BPC * R            # 784 packed img tokens per core
WSCALE = 64.0
F32 = mybir.dt.float32
BF16 = mybir.dt.bfloat16
F8E4 = mybir.dt.float8e4
F8E3 = mybir.dt.float8e3
AF = mybir.ActivationFunctionType
DR = mybir.MatmulPerfMode.DoubleRow

_CACHE = {}


def _build():
    nc = bacc.Bacc("TRN2", target_bir_lowering=False, debug=False,
                   num_devices=NCORES)
    d = {
        "w8t": nc.dram_tensor("w8t", [P, NT * KO * 2 * P], F8E4,
                              kind="ExternalInput").ap(),
        "w8i": nc.dram_tensor("w8i", [P, NT * KO * 2 * P], F8E4,
                              kind="ExternalInput").ap(),
        "xt8t": nc.dram_tensor("xt8t", [P, KO * 2 * TTOK], F8E4,
                               kind="ExternalInput").ap(),
        "xt8i": nc.dram_tensor("xt8i", [P, KO * 2 * ITOKP], F8E4,
                               kind="ExternalInput").ap(),
        "wcs": nc.dram_tensor("wcs", [P, 2 * NT], BF16,
                              kind="ExternalInput").ap(),
        "tnat": nc.dram_tensor("tnat", [TTOK, D + 1], F8E3,
                               kind="ExternalInput").ap(),
        "inat": nc.dram_tensor("inat", [2 * BPC * P, D + 1], F8E3,
                               kind="ExternalInput").ap(),
        "u_out": nc.dram_tensor("u_out", [1, BPC * (D + 1)], F32,
                                kind="ExternalOutput").ap(),
        "v_out": nc.dram_tensor("v_out", [1, BPC * (D + 1)], F32,
                                kind="ExternalOutput").ap(),
    }
    with tile.TileContext(nc) as tc:
        _emit(tc, d)
    nc.compile()
    return nc


def _emit(tc, d):
    from contextlib import ExitStack

    nc = tc.nc
    with ExitStack() as ctx:
        wpool = ctx.enter_context(tc.tile_pool(name="w", bufs=1))
        xpool = ctx.enter_context(tc.tile_pool(name="x", bufs=1))
        cpool = ctx.enter_context(tc.tile_pool(name="c", bufs=1))
        thpool = ctx.enter_context(tc.tile_pool(name="th", bufs=3))
        qpool = ctx.enter_context(tc.tile_pool(name="q", bufs=1))
        opool = ctx.enter_context(tc.tile_pool(name="o", bufs=1))
        pm = ctx.enter_context(tc.tile_pool(name="pm", bufs=2, space="PSUM"))
        pq = ctx.enter_context(tc.tile_pool(name="pq", bufs=1, space="PSUM"))
        pu = ctx.enter_context(tc.tile_pool(name="pu", bufs=3, space="PSUM"))

        # ---- input DMAs, consumption order: text-sc0, img ph1, text
        # naturals b0/b1, text-sc1, img naturals, text naturals b2/b3 ----
        w8i = wpool.tile([P, NT, KO, 2, P], F8E4)
        w8i_r = d["w8i"].rearrange("p (n g i c) -> p n g i c",
                                   n=NT, g=KO, i=2)
        nc.sync.dma_start(w8i[:, 0:2], w8i_r[:, 0:2])
        xt8i = xpool.tile([P, KO, 2, ITOKP], F8E4)
        xt8i_r = d["xt8i"].rearrange("p (g i t) -> p g i t", g=KO, i=2)
        nc.sync.dma_start(xt8i[:, :, :, 0:512], xt8i_r[:, :, :, 0:512])
        nc.sync.dma_start(xt8i[:, :, :, 512:ITOKP], xt8i_r[:, :, :, 512:ITOKP])
        nc.sync.dma_start(w8i[:, 2:NT], w8i_r[:, 2:NT])
        w8t = wpool.tile([P, NT, KO, 2, P], F8E4)
        w8t_r = d["w8t"].rearrange("p (n g i c) -> p n g i c",
                                   n=NT, g=KO, i=2)
        nc.sync.dma_start(w8t[:], w8t_r[:])
        xt8t = xpool.tile([P, KO, 2, TTOK], F8E4)
        xt8t_r = d["xt8t"].rearrange("p (g i t) -> p g i t", g=KO, i=2)
        nc.sync.dma_start(xt8t[:, :, :, 0:1024], xt8t_r[:, :, :, 0:1024])
        wcs = cpool.tile([P, 2, NT], BF16)
        nc.sync.dma_start(wcs[:], d["wcs"].rearrange("p (a n) -> p a n", a=2))
        tnat = xpool.tile([P, 4 * BPC, D + 1], F8E3)
        tnat_r = d["tnat"].rearrange("(t p) n -> p t n", p=P)
        nc.sync.dma_start(tnat[:, 0:8, :], tnat_r[:, 0:8, :])
        inat = xpool.tile([P, 2 * BPC, D + 1], F8E3)
        nc.sync.dma_start(inat[:],
                          d["inat"].rearrange("(t p) n -> p t n", p=P))
        nc.sync.dma_start(xt8t[:, :, :, 1024:2048], xt8t_r[:, :, :, 1024:2048])
        nc.sync.dma_start(tnat[:, 8:16, :], tnat_r[:, 8:16, :])

        u_sb = opool.tile([1, BPC, D + 1], F32)
        v_sb = opool.tile([1, BPC, D + 1], F32)
        qst_t = qpool.tile([P, 16], BF16)
        qst_i = qpool.tile([P, 8], BF16)

        def ph1(w8, xt8, tok0, ntok, th_tag, split_tanh=0):
            """Feature-major fp8-DR matmuls + tanh into a bf16 tile.
            split_tanh: tanh the first k n-tiles per token-half so ACT can
            start before the second half's inputs land."""
            halves = [(o, min(512, ntok - o)) for o in range(0, ntok, 512)]
            th8 = thpool.tile([P, NT, 1024], BF16, tag=th_tag)
            for n in range(NT):
                mp = pm.tile([P, 1024], F32, tag="pm")
                for g in range(KO):
                    for off, sz in halves:
                        nc.tensor.matmul(
                            mp[:, off:off + sz],
                            lhsT=w8[:, n, g],
                            rhs=xt8[:, g, :, tok0 + off:tok0 + off + sz],
                            start=(g == 0), stop=(g == KO - 1),
                            perf_mode=DR)
                if n < split_tanh:
                    for off, sz in halves:
                        nc.scalar.activation(th8[:, n, off:off + sz],
                                             mp[:, off:off + sz],
                                             AF.Tanh, scale=1.0 / WSCALE)
                else:
                    nc.scalar.activation(th8[:, n, 0:ntok], mp[:, 0:ntok],
                                         AF.Tanh, scale=1.0 / WSCALE)
            return th8

        def dexp(th8, wci, qst, qbase, slices, ntok_pad=False):
            """d-score columns (groups on one PSUM bank must stay contiguous:
            start=True clears the whole bank's has_written), then exp."""
            nsl = len(slices)
            qcolp = pq.tile([P, 8], F32, tag="qc")
            if ntok_pad:
                nc.vector.memset(qcolp[:], 0.0)
            for s, (c0, w) in enumerate(slices):
                for n in range(NT):
                    nc.tensor.matmul(
                        qcolp[0:w, s:s + 1],
                        lhsT=th8[:, n, c0:c0 + w],
                        rhs=wcs[:, wci, n:n + 1],
                        start=(n == 0), stop=(n == NT - 1))
            nc.scalar.activation(qst[:, qbase:qbase + nsl], qcolp[:, 0:nsl],
                                 AF.Exp, scale=1.0 / WSCALE)

        def wsum(qst, qbase, nat, sb_stage, ws_tiles, bat0, tail=False):
            """fp8e3 weighted sums + inline ones-column denominator, one
            single-bank PSUM accumulator per (batch, feature-half)."""
            per_b = {}
            for s, (b_local, nat_tile, rows) in enumerate(ws_tiles):
                per_b.setdefault(b_local, []).append((s, nat_tile, rows))
            for b_local, tiles in per_b.items():
                bb = bat0 + b_local
                for off, sz in ((0, 512), (512, D + 1 - 512)):
                    ups = pu.tile([1, 512], F32, tag="ups")
                    for ci, (s, nat_tile, rows) in enumerate(tiles):
                        nc.tensor.matmul(
                            ups[:1, 0:sz],
                            lhsT=qst[0:rows, qbase + s:qbase + s + 1],
                            rhs=nat[0:rows, nat_tile, off:off + sz],
                            start=(ci == 0), stop=(ci == len(tiles) - 1))
                    if tail and off:   # ACT is drained on the tail phase
                        nc.scalar.activation(sb_stage[:1, bb, off:off + sz],
                                             ups[:1, 0:sz], AF.Copy)
                    else:
                        nc.vector.tensor_copy(sb_stage[:1, bb, off:off + sz],
                                              ups[:1, 0:sz])

        # PE stream: sc0-ph1, img-ph1 (fills sc0 tanh tail), sc0 d/ws,
        # sc1-ph1, img d/ws, sc1 d/ws (tail)
        t_slices0 = [(128 * s, P) for s in range(8)]
        i_slices = [(R * b + h * P, P if h == 0 else R - P)
                    for b in range(BPC) for h in range(2)]
        ws_img = [(b, 2 * b + h, P if h == 0 else R - P)
                  for b in range(BPC) for h in range(2)]
        th_img = ph1(w8i, xt8i, 0, ITOKP, "th", split_tanh=2)
        dexp(th_img, 1, qst_i, 0, i_slices, ntok_pad=True)
        th_sc0 = ph1(w8t, xt8t, 0, 1024, "th")
        wsum(qst_i, 0, inat, v_sb, ws_img, 0)
        nc.sync.dma_start(
            d["v_out"].rearrange("p (b n) -> p b n", b=BPC), v_sb[:, :, :])
        dexp(th_sc0, 0, qst_t, 0, t_slices0)
        th_sc1 = ph1(w8t, xt8t, 1024, 1024, "th")
        wsum(qst_t, 0, tnat, u_sb, [(s // 4, s, P) for s in range(8)], 0)
        dexp(th_sc1, 0, qst_t, 8, t_slices0)
        wsum(qst_t, 8, tnat, u_sb, [(s // 4, 8 + s, P) for s in range(8)], 2,
             tail=True)
        nc.sync.dma_start(
            d["u_out"].rearrange("p (b n) -> p b n", b=BPC), u_sb[:, :, :])


def _get_nc():
    if "nc" not in _CACHE:
        _CACHE["nc"] = _build()
    return _CACHE["nc"]


def prep_core_inputs(inputs):
    """Host-side prep: slice per core, pack img tokens, cast, pre-arrange."""
    e3 = ml_dtypes.float8_e3m4
    e4 = ml_dtypes.float8_e4m3
    text = np.asarray(inputs["text_features"], dtype=np.float32)
    img = np.asarray(inputs["img_features"], dtype=np.float32)

    def warr(w):  # [D, D] -> [P, NT*KO*2*P] n-major chunks
        a = (np.asarray(w, np.float32) * WSCALE).astype(e4)
        # a[g*256 + i*128 + p, n*128 + c] -> out[p, n, g, i, c]
        return np.ascontiguousarray(
            a.reshape(KO, 2, P, NT, P).transpose(2, 3, 0, 1, 4)
        ).reshape(P, NT * KO * 2 * P)

    def wcol(w):  # wa[D:] -> [P, NT] bf16 columns (x64)
        ws = np.asarray(w, np.float32)[D:] * WSCALE
        return np.ascontiguousarray(
            ws.reshape(NT, P).T).astype(ml_dtypes.bfloat16)

    def xtarr(x):  # [T, D] -> feature-major DR layout [P, KO*2*T]
        t = x.shape[0]
        a = np.ascontiguousarray(x.T).astype(e4)   # [D, T]
        return np.ascontiguousarray(
            a.reshape(KO, 2, P, t).transpose(2, 0, 1, 3)).reshape(P, -1)

    w8t = warr(inputs["Wt2"])
    w8i = warr(inputs["Wi1"])
    wcs = np.concatenate([wcol(inputs["wa2"]), wcol(inputs["wa1"])], axis=1)

    in_maps = []
    ones_t = np.ones((TTOK, 1), np.float32)
    ones_i = np.ones((ITOKP, 1), np.float32)
    for c in range(NCORES):
        tc = text[BPC * c:BPC * (c + 1)].reshape(TTOK, D)
        ic = img[BPC * c:BPC * (c + 1)].reshape(ITOKP, D)
        inat = np.zeros((2 * BPC * P, D + 1), e3)
        ic1 = np.hstack([ic, ones_i]).astype(e3)
        for b in range(BPC):
            inat[2 * b * P:2 * b * P + R] = ic1[R * b:R * (b + 1)]
        in_maps.append({
            "w8t": w8t, "w8i": w8i, "wcs": wcs,
            "xt8t": xtarr(tc), "xt8i": xtarr(ic),
            "tnat": np.hstack([tc, ones_t]).astype(e3),
            "inat": inat,
        })
    return in_maps


def kernel(**inputs):
    nc = _get_nc()
    in_maps = prep_core_inputs(inputs)
    res = run_bass_kernel_spmd(nc, in_maps, list(range(NCORES)))
    u = np.concatenate([res.results[c]["u_out"].reshape(BPC, D + 1)
                        for c in range(NCORES)], axis=0)
    v = np.concatenate([res.results[c]["v_out"].reshape(BPC, D + 1)
                        for c in range(NCORES)], axis=0)
    u = u[:, :D] / u[:, D:D + 1]
    v = v[:, :D] / v[:, D:D + 1]
    att_text = np.broadcast_to(u[:, None, :], (B, S, D)).astype(np.float32).copy()
    att_img = np.broadcast_to(v[:, None, :], (B, S, D)).astype(np.float32).copy()
    return att_text, att_img


# revision 3
# speedup vs baseline: 1.0045x; 1.0045x over previous
"""CoAttention kernel for Trainium2, 8 NeuronCores, pure data parallel.

Math shortcut (exact, softmax shift-invariance): scores are additive in the
query index, so attention weights are query-independent:
    u[b] = softmax_j(tanh(text[b] @ Wt2) @ wa2[D:]) @ text[b]   (text out row)
    v[b] = softmax_r(tanh(img[b]  @ Wi1) @ wa1[D:]) @ img[b]    (img out row)
Wt1/bt1/Wi2/bi2/wa1[:D]/wa2[:D]/ba1/ba2 cancel exactly; host broadcasts over S.

Performance design (cost-model driven):
- ph1 (the big X@W matmuls) runs feature-major (W stationary, X^T moving) in
  fp8e4m3 DoubleRow: 0.5 cyc/col.  W pre-scaled x64 on host (fp8 subnormal
  dodge), 1/64 folded into tanh/exp activation scales.
- d = w.tanh(Y) is computed as matmul COLUMNS: lhsT = tanh tile (stationary,
  fp8 out of ACT), rhs = w column (fp8 DR) -> out [128 tok, 1].  Output free
  size 1 => near-zero engine time, and the scores land token-on-partition for
  the weighted sum directly: no d-row copies, no PE transposes.
- exp on ACT straight from the d-column PSUM tile, one call per superchunk,
  bf16 out (PE accepts mixed-dtype operands: bf16 lhsT x fp8 rhs).
- Weighted sums use fp8e3m4 token-major naturals (1.0 cyc/col, HALF the DMA
  bytes of bf16; e3m4 on N(0,1) has 1.3% RMS quant err -> ~4e-3 added absmax,
  inside the 2e-2 gate).  The 769th all-ones column produces the softmax
  denominator inside the same matmul; the final divide happens on HOST
  (outputs ship as numerator+denominator rows).
- img tokens ship PACKED (196/batch, no 256-pad): partial 68-row tiles are
  partition-sliced matmuls; junk PSUM rows are memset-zeroed once.
- Per-batch ups PSUM row is copied out split DVE(0:512)+Pool(512:769) so the
  single ups bank frees fast.  PSUM budget: ph1 2x[P,1024](4) + qcol 2 +
  ups 2 = 8 banks exactly.
"""

import numpy as np
import ml_dtypes

import concourse.bacc as bacc
import concourse.mybir as mybir
import concourse.tile as tile
from concourse.bass_utils import run_bass_kernel_spmd

B, S, R, D = 32, 512, 196, 768
NCORES = 8
BPC = B // NCORES          # 4 batches per core
P = 128
KO = 3                     # DoubleRow contraction groups of 256
NT = D // P                # 6 feature tiles
TTOK = BPC * S             # 2048 text tokens per core
ITOKP = BPC * R            # 784 packed img tokens per core
WSCALE = 64.0
F32 = mybir.dt.float32
BF16 = mybir.dt.bfloat16
F8E4 = mybir.dt.float8e4
F8E3 = mybir.dt.float8e3
AF = mybir.ActivationFunctionType
DR = mybir.MatmulPerfMode.DoubleRow

_CACHE = {}


def _build():
    nc = bacc.Bacc("TRN2", target_bir_lowering=False, debug=False,
                   num_devices=NCORES)
    d = {
        "w8t": nc.dram_tensor("w8t", [P, NT * KO * 2 * P], F8E4,
                              kind="ExternalInput").ap(),
        "w8i": nc.dram_tensor("w8i", [P, NT * KO * 2 * P], F8E4,
                              kind="ExternalInput").ap(),
        "xt8t": nc.dram_tensor("xt8t", [P, KO * 2 * TTOK], F8E4,
                               kind="ExternalInput").ap(),
        "xt8i": nc.dram_tensor("xt8i", [P, KO * 2 * ITOKP], F8E4,
                               kind="ExternalInput").ap(),
        "wcs": nc.dram_tensor("wcs", [P, 2 * NT], BF16,
                              kind="ExternalInput").ap(),
        "tnat": nc.dram_tensor("tnat", [TTOK, D + 1], F8E3,
                               kind="ExternalInput").ap(),
        "inat": nc.dram_tensor("inat", [2 * BPC * P, D + 1], F8E3,
                               kind="ExternalInput").ap(),
        "u_out": nc.dram_tensor("u_out", [1, BPC * (D + 1)], F32,
                                kind="ExternalOutput").ap(),
        "v_out": nc.dram_tensor("v_out", [1, BPC * (D + 1)], F32,
                                kind="ExternalOutput").ap(),
    }
    with tile.TileContext(nc) as tc:
        _emit(tc, d)
    nc.compile()
    return nc


def _emit(tc, d):
    from contextlib import ExitStack

    nc = tc.nc
    with ExitStack() as ctx:
        wpool = ctx.enter_context(tc.tile_pool(name="w", bufs=1))
        xpool = ctx.enter_context(tc.tile_pool(name="x", bufs=1))
        cpool = ctx.enter_context(tc.tile_pool(name="c", bufs=1))
        thpool = ctx.enter_context(tc.tile_pool(name="th", bufs=3))
        qpool = ctx.enter_context(tc.tile_pool(name="q", bufs=1))
        opool = ctx.enter_context(tc.tile_pool(name="o", bufs=1))
        pm = ctx.enter_context(tc.tile_pool(name="pm", bufs=2, space="PSUM"))
        pq = ctx.enter_context(tc.tile_pool(name="pq", bufs=1, space="PSUM"))
        pu = ctx.enter_context(tc.tile_pool(name="pu", bufs=3, space="PSUM"))

        # ---- input DMAs, consumption order: text-sc0, img ph1, text
        # naturals b0/b1, text-sc1, img naturals, text naturals b2/b3 ----
        w8i = wpool.tile([P, NT, KO, 2, P], F8E4)
        w8i_r = d["w8i"].rearrange("p (n g i c) -> p n g i c",
                                   n=NT, g=KO, i=2)
        nc.sync.dma_start(w8i[:, 0:2], w8i_r[:, 0:2])
        xt8i = xpool.tile([P, KO, 2, ITOKP], F8E4)
        xt8i_r = d["xt8i"].rearrange("p (g i t) -> p g i t", g=KO, i=2)
        nc.sync.dma_start(xt8i[:, :, :, 0:512], xt8i_r[:, :, :, 0:512])
        nc.sync.dma_start(xt8i[:, :, :, 512:ITOKP], xt8i_r[:, :, :, 512:ITOKP])
        nc.sync.dma_start(w8i[:, 2:NT], w8i_r[:, 2:NT])
        w8t = wpool.tile([P, NT, KO, 2, P], F8E4)
        w8t_r = d["w8t"].rearrange("p (n g i c) -> p n g i c",
                                   n=NT, g=KO, i=2)
        nc.sync.dma_start(w8t[:], w8t_r[:])
        xt8t = xpool.tile([P, KO, 2, TTOK], F8E4)
        xt8t_r = d["xt8t"].rearrange("p (g i t) -> p g i t", g=KO, i=2)
        nc.sync.dma_start(xt8t[:, :, :, 0:1024], xt8t_r[:, :, :, 0:1024])
        wcs = cpool.tile([P, 2, NT], BF16)
        nc.sync.dma_start(wcs[:], d["wcs"].rearrange("p (a n) -> p a n", a=2))
        tnat = xpool.tile([P, 4 * BPC, D + 1], F8E3)
        tnat_r = d["tnat"].rearrange("(t p) n -> p t n", p=P)
        nc.sync.dma_start(tnat[:, 0:8, :], tnat_r[:, 0:8, :])
        inat = xpool.tile([P, 2 * BPC, D + 1], F8E3)
        nc.sync.dma_start(inat[:],
                          d["inat"].rearrange("(t p) n -> p t n", p=P))
        nc.sync.dma_start(xt8t[:, :, :, 1024:2048], xt8t_r[:, :, :, 1024:2048])
        nc.sync.dma_start(tnat[:, 8:16, :], tnat_r[:, 8:16, :])

        u_sb = opool.tile([1, BPC, D + 1], F32)
        v_sb = opool.tile([1, BPC, D + 1], F32)
        qst_t = qpool.tile([P, 16], BF16)
        qst_i = qpool.tile([P, 8], BF16)

        def ph1(w8, xt8, tok0, ntok, th_tag, split_tanh=0):
            """Feature-major fp8-DR matmuls + tanh into a bf16 tile.
            split_tanh: tanh the first k n-tiles per token-half so ACT can
            start before the second half's inputs land."""
            halves = [(o, min(512, ntok - o)) for o in range(0, ntok, 512)]
            th8 = thpool.tile([P, NT, 1024], BF16, tag=th_tag)
            for n in range(NT):
                mp = pm.tile([P, 1024], F32, tag="pm")
                for off, sz in halves:
                    for g in range(KO):
                        nc.tensor.matmul(
                            mp[:, off:off + sz],
                            lhsT=w8[:, n, g],
                            rhs=xt8[:, g, :, tok0 + off:tok0 + off + sz],
                            start=(g == 0), stop=(g == KO - 1),
                            perf_mode=DR)
                if n < split_tanh:
                    for off, sz in halves:
                        nc.scalar.activation(th8[:, n, off:off + sz],
                                             mp[:, off:off + sz],
                                             AF.Tanh, scale=1.0 / WSCALE)
                else:
                    nc.scalar.activation(th8[:, n, 0:ntok], mp[:, 0:ntok],
                                         AF.Tanh, scale=1.0 / WSCALE)
            return th8

        def dexp(th8, wci, qst, qbase, slices, ntok_pad=False):
            """d-score columns (groups on one PSUM bank must stay contiguous:
            start=True clears the whole bank's has_written), then exp."""
            nsl = len(slices)
            qcolp = pq.tile([P, 8], F32, tag="qc")
            if ntok_pad:
                nc.vector.memset(qcolp[:], 0.0)
            for s, (c0, w) in enumerate(slices):
                for n in range(NT):
                    nc.tensor.matmul(
                        qcolp[0:w, s:s + 1],
                        lhsT=th8[:, n, c0:c0 + w],
                        rhs=wcs[:, wci, n:n + 1],
                        start=(n == 0), stop=(n == NT - 1))
            nc.scalar.activation(qst[:, qbase:qbase + nsl], qcolp[:, 0:nsl],
                                 AF.Exp, scale=1.0 / WSCALE)

        def wsum(qst, qbase, nat, sb_stage, ws_tiles, bat0, tail=False):
            """fp8e3 weighted sums + inline ones-column denominator, one
            single-bank PSUM accumulator per (batch, feature-half)."""
            per_b = {}
            for s, (b_local, nat_tile, rows) in enumerate(ws_tiles):
                per_b.setdefault(b_local, []).append((s, nat_tile, rows))
            for b_local, tiles in per_b.items():
                bb = bat0 + b_local
                for off, sz in ((0, 512), (512, D + 1 - 512)):
                    ups = pu.tile([1, 512], F32, tag="ups")
                    for ci, (s, nat_tile, rows) in enumerate(tiles):
                        nc.tensor.matmul(
                            ups[:1, 0:sz],
                            lhsT=qst[0:rows, qbase + s:qbase + s + 1],
                            rhs=nat[0:rows, nat_tile, off:off + sz],
                            start=(ci == 0), stop=(ci == len(tiles) - 1))
                    if tail and off:   # ACT is drained on the tail phase
                        nc.scalar.activation(sb_stage[:1, bb, off:off + sz],
                                             ups[:1, 0:sz], AF.Copy)
                    else:
                        nc.vector.tensor_copy(sb_stage[:1, bb, off:off + sz],
                                              ups[:1, 0:sz])

        # PE stream: sc0-ph1, img-ph1 (fills sc0 tanh tail), sc0 d/ws,
        # sc1-ph1, img d/ws, sc1 d/ws (tail)
        t_slices0 = [(128 * s, P) for s in range(8)]
        i_slices = [(R * b + h * P, P if h == 0 else R - P)
                    for b in range(BPC) for h in range(2)]
        ws_img = [(b, 2 * b + h, P if h == 0 else R - P)
                  for b in range(BPC) for h in range(2)]
        th_img = ph1(w8i, xt8i, 0, ITOKP, "th", split_tanh=2)
        dexp(th_img, 1, qst_i, 0, i_slices, ntok_pad=True)
        th_sc0 = ph1(w8t, xt8t, 0, 1024, "th")
        wsum(qst_i, 0, inat, v_sb, ws_img, 0)
        nc.sync.dma_start(
            d["v_out"].rearrange("p (b n) -> p b n", b=BPC), v_sb[:, :, :])
        dexp(th_sc0, 0, qst_t, 0, t_slices0)
        th_sc1 = ph1(w8t, xt8t, 1024, 1024, "th")
        wsum(qst_t, 0, tnat, u_sb, [(s // 4, s, P) for s in range(8)], 0)
        dexp(th_sc1, 0, qst_t, 8, t_slices0)
        wsum(qst_t, 8, tnat, u_sb, [(s // 4, 8 + s, P) for s in range(8)], 2,
             tail=True)
        nc.sync.dma_start(
            d["u_out"].rearrange("p (b n) -> p b n", b=BPC), u_sb[:, :, :])


def _get_nc():
    if "nc" not in _CACHE:
        _CACHE["nc"] = _build()
    return _CACHE["nc"]


def prep_core_inputs(inputs):
    """Host-side prep: slice per core, pack img tokens, cast, pre-arrange."""
    e3 = ml_dtypes.float8_e3m4
    e4 = ml_dtypes.float8_e4m3
    text = np.asarray(inputs["text_features"], dtype=np.float32)
    img = np.asarray(inputs["img_features"], dtype=np.float32)

    def warr(w):  # [D, D] -> [P, NT*KO*2*P] n-major chunks
        a = (np.asarray(w, np.float32) * WSCALE).astype(e4)
        # a[g*256 + i*128 + p, n*128 + c] -> out[p, n, g, i, c]
        return np.ascontiguousarray(
            a.reshape(KO, 2, P, NT, P).transpose(2, 3, 0, 1, 4)
        ).reshape(P, NT * KO * 2 * P)

    def wcol(w):  # wa[D:] -> [P, NT] bf16 columns (x64)
        ws = np.asarray(w, np.float32)[D:] * WSCALE
        return np.ascontiguousarray(
            ws.reshape(NT, P).T).astype(ml_dtypes.bfloat16)

    def xtarr(x):  # [T, D] -> feature-major DR layout [P, KO*2*T]
        t = x.shape[0]
        a = np.ascontiguousarray(x.T).astype(e4)   # [D, T]
        return np.ascontiguousarray(
            a.reshape(KO, 2, P, t).transpose(2, 0, 1, 3)).reshape(P, -1)

    w8t = warr(inputs["Wt2"])
    w8i = warr(inputs["Wi1"])
    wcs = np.concatenate([wcol(inputs["wa2"]), wcol(inputs["wa1"])], axis=1)

    in_maps = []
    ones_t = np.ones((TTOK, 1), np.float32)
    ones_i = np.ones((ITOKP, 1), np.float32)
    for c in range(NCORES):
        tc = text[BPC * c:BPC * (c + 1)].reshape(TTOK, D)
        ic = img[BPC * c:BPC * (c + 1)].reshape(ITOKP, D)
        inat = np.zeros((2 * BPC * P, D + 1), e3)
        ic1 = np.hstack([ic, ones_i]).astype(e3)
        for b in range(BPC):
            inat[2 * b * P:2 * b * P + R] = ic1[R * b:R * (b + 1)]
        in_maps.append({
            "w8t": w8t, "w8i": w8i, "wcs": wcs,
            "xt8t": xtarr(tc), "xt8i": xtarr(ic),
            "tnat": np.hstack([tc, ones_t]).astype(e3),
            "inat": inat,
        })
    return in_maps


def kernel(**inputs):
    nc = _get_nc()
    in_maps = prep_core_inputs(inputs)
    res = run_bass_kernel_spmd(nc, in_maps, list(range(NCORES)))
    u = np.concatenate([res.results[c]["u_out"].reshape(BPC, D + 1)
                        for c in range(NCORES)], axis=0)
    v = np.concatenate([res.results[c]["v_out"].reshape(BPC, D + 1)
                        for c in range(NCORES)], axis=0)
    u = u[:, :D] / u[:, D:D + 1]
    v = v[:, :D] / v[:, D:D + 1]
    att_text = np.broadcast_to(u[:, None, :], (B, S, D)).astype(np.float32).copy()
    att_img = np.broadcast_to(v[:, None, :], (B, S, D)).astype(np.float32).copy()
    return att_text, att_img


# revision 4
# speedup vs baseline: 1.0046x; 1.0001x over previous
"""CoAttention kernel for Trainium2, 8 NeuronCores, pure data parallel.

Math shortcut (exact, softmax shift-invariance): scores are additive in the
query index, so attention weights are query-independent:
    u[b] = softmax_j(tanh(text[b] @ Wt2) @ wa2[D:]) @ text[b]   (text out row)
    v[b] = softmax_r(tanh(img[b]  @ Wi1) @ wa1[D:]) @ img[b]    (img out row)
Wt1/bt1/Wi2/bi2/wa1[:D]/wa2[:D]/ba1/ba2 cancel exactly; host broadcasts over S.

Performance design (cost-model driven):
- ph1 (the big X@W matmuls) runs feature-major (W stationary, X^T moving) in
  fp8e4m3 DoubleRow: 0.5 cyc/col.  W pre-scaled x64 on host (fp8 subnormal
  dodge), 1/64 folded into tanh/exp activation scales.
- d = w.tanh(Y) is computed as matmul COLUMNS: lhsT = tanh tile (stationary,
  fp8 out of ACT), rhs = w column (fp8 DR) -> out [128 tok, 1].  Output free
  size 1 => near-zero engine time, and the scores land token-on-partition for
  the weighted sum directly: no d-row copies, no PE transposes.
- exp on ACT straight from the d-column PSUM tile, one call per superchunk,
  bf16 out (PE accepts mixed-dtype operands: bf16 lhsT x fp8 rhs).
- Weighted sums use fp8e3m4 token-major naturals (1.0 cyc/col, HALF the DMA
  bytes of bf16; e3m4 on N(0,1) has 1.3% RMS quant err -> ~4e-3 added absmax,
  inside the 2e-2 gate).  The 769th all-ones column produces the softmax
  denominator inside the same matmul; the final divide happens on HOST
  (outputs ship as numerator+denominator rows).
- img tokens ship PACKED (196/batch, no 256-pad): partial 68-row tiles are
  partition-sliced matmuls; junk PSUM rows are memset-zeroed once.
- Per-batch ups PSUM row is copied out split DVE(0:512)+Pool(512:769) so the
  single ups bank frees fast.  PSUM budget: ph1 2x[P,1024](4) + qcol 2 +
  ups 2 = 8 banks exactly.
"""

import numpy as np
import ml_dtypes

import concourse.bacc as bacc
import concourse.mybir as mybir
import concourse.tile as tile
from concourse.bass_utils import run_bass_kernel_spmd

B, S, R, D = 32, 512, 196, 768
NCORES = 8
BPC = B // NCORES          # 4 batches per core
P = 128
KO = 3                     # DoubleRow contraction groups of 256
NT = D // P                # 6 feature tiles
TTOK = BPC * S             # 2048 text tokens per core
ITOKP = BPC * R            # 784 packed img tokens per core
WSCALE = 64.0
F32 = mybir.dt.float32
BF16 = mybir.dt.bfloat16
F8E4 = mybir.dt.float8e4
F8E3 = mybir.dt.float8e3
AF = mybir.ActivationFunctionType
DR = mybir.MatmulPerfMode.DoubleRow

_CACHE = {}


def _build():
    nc = bacc.Bacc("TRN2", target_bir_lowering=False, debug=False,
                   num_devices=NCORES)
    d = {
        "w8t": nc.dram_tensor("w8t", [P, NT * KO * 2 * P], F8E4,
                              kind="ExternalInput").ap(),
        "w8i": nc.dram_tensor("w8i", [P, NT * KO * 2 * P], F8E4,
                              kind="ExternalInput").ap(),
        "xt8t": nc.dram_tensor("xt8t", [P, KO * 2 * TTOK], F8E4,
                               kind="ExternalInput").ap(),
        "xt8i": nc.dram_tensor("xt8i", [P, KO * 2 * ITOKP], F8E4,
                               kind="ExternalInput").ap(),
        "wcs": nc.dram_tensor("wcs", [P, 2 * NT], BF16,
                              kind="ExternalInput").ap(),
        "tnat": nc.dram_tensor("tnat", [TTOK, D + 1], F8E3,
                               kind="ExternalInput").ap(),
        "inat": nc.dram_tensor("inat", [2 * BPC * P, D + 1], F8E3,
                               kind="ExternalInput").ap(),
        "u_out": nc.dram_tensor("u_out", [1, BPC * (D + 1)], F32,
                                kind="ExternalOutput").ap(),
        "v_out": nc.dram_tensor("v_out", [1, BPC * (D + 1)], F32,
                                kind="ExternalOutput").ap(),
    }
    with tile.TileContext(nc) as tc:
        _emit(tc, d)
    nc.compile()
    return nc


def _emit(tc, d):
    from contextlib import ExitStack

    nc = tc.nc
    with ExitStack() as ctx:
        wpool = ctx.enter_context(tc.tile_pool(name="w", bufs=1))
        xpool = ctx.enter_context(tc.tile_pool(name="x", bufs=1))
        cpool = ctx.enter_context(tc.tile_pool(name="c", bufs=1))
        thpool = ctx.enter_context(tc.tile_pool(name="th", bufs=3))
        qpool = ctx.enter_context(tc.tile_pool(name="q", bufs=1))
        opool = ctx.enter_context(tc.tile_pool(name="o", bufs=1))
        pm = ctx.enter_context(tc.tile_pool(name="pm", bufs=2, space="PSUM"))
        pq = ctx.enter_context(tc.tile_pool(name="pq", bufs=1, space="PSUM"))
        pu = ctx.enter_context(tc.tile_pool(name="pu", bufs=3, space="PSUM"))

        # ---- input DMAs, consumption order: text-sc0, img ph1, text
        # naturals b0/b1, text-sc1, img naturals, text naturals b2/b3 ----
        w8i = wpool.tile([P, NT, KO, 2, P], F8E4)
        w8i_r = d["w8i"].rearrange("p (n g i c) -> p n g i c",
                                   n=NT, g=KO, i=2)
        nc.sync.dma_start(w8i[:, 0:2], w8i_r[:, 0:2])
        xt8i = xpool.tile([P, KO, 2, ITOKP], F8E4)
        xt8i_r = d["xt8i"].rearrange("p (g i t) -> p g i t", g=KO, i=2)
        nc.sync.dma_start(xt8i[:, :, :, 0:512], xt8i_r[:, :, :, 0:512])
        nc.sync.dma_start(xt8i[:, :, :, 512:ITOKP], xt8i_r[:, :, :, 512:ITOKP])
        nc.sync.dma_start(w8i[:, 2:NT], w8i_r[:, 2:NT])
        w8t = wpool.tile([P, NT, KO, 2, P], F8E4)
        w8t_r = d["w8t"].rearrange("p (n g i c) -> p n g i c",
                                   n=NT, g=KO, i=2)
        nc.sync.dma_start(w8t[:], w8t_r[:])
        xt8t = xpool.tile([P, KO, 2, TTOK], F8E4)
        xt8t_r = d["xt8t"].rearrange("p (g i t) -> p g i t", g=KO, i=2)
        nc.sync.dma_start(xt8t[:, :, :, 0:1024], xt8t_r[:, :, :, 0:1024])
        wcs = cpool.tile([P, 2, NT], BF16)
        nc.sync.dma_start(wcs[:], d["wcs"].rearrange("p (a n) -> p a n", a=2))
        tnat = xpool.tile([P, 4 * BPC, D + 1], F8E3)
        tnat_r = d["tnat"].rearrange("(t p) n -> p t n", p=P)
        nc.sync.dma_start(tnat[:, 0:8, :], tnat_r[:, 0:8, :])
        inat = xpool.tile([P, 2 * BPC, D + 1], F8E3)
        nc.sync.dma_start(inat[:],
                          d["inat"].rearrange("(t p) n -> p t n", p=P))
        nc.sync.dma_start(xt8t[:, :, :, 1024:2048], xt8t_r[:, :, :, 1024:2048])
        nc.sync.dma_start(tnat[:, 8:16, :], tnat_r[:, 8:16, :])

        u_sb = opool.tile([1, BPC, D + 1], F32)
        v_sb = opool.tile([1, BPC, D + 1], F32)
        qst_t = qpool.tile([P, 16], BF16)
        qst_i = qpool.tile([P, 8], BF16)

        def ph1(w8, xt8, tok0, ntok, th_tag, split_tanh=0):
            """Feature-major fp8-DR matmuls + tanh into a bf16 tile.
            split_tanh: tanh the first k n-tiles per token-half so ACT can
            start before the second half's inputs land."""
            halves = [(o, min(512, ntok - o)) for o in range(0, ntok, 512)]
            th8 = thpool.tile([P, NT, 1024], BF16, tag=th_tag)
            for n in range(NT):
                mp = pm.tile([P, 1024], F32, tag="pm")
                for off, sz in halves:
                    for g in range(KO):
                        nc.tensor.matmul(
                            mp[:, off:off + sz],
                            lhsT=w8[:, n, g],
                            rhs=xt8[:, g, :, tok0 + off:tok0 + off + sz],
                            start=(g == 0), stop=(g == KO - 1),
                            perf_mode=DR)
                if n < split_tanh:
                    for off, sz in halves:
                        nc.scalar.activation(th8[:, n, off:off + sz],
                                             mp[:, off:off + sz],
                                             AF.Tanh, scale=1.0 / WSCALE)
                else:
                    nc.scalar.activation(th8[:, n, 0:ntok], mp[:, 0:ntok],
                                         AF.Tanh, scale=1.0 / WSCALE)
            return th8

        def dexp(th8, wci, qst, qbase, slices, ntok_pad=False,
                 split_exp=False):
            """d-score columns (groups on one PSUM bank must stay contiguous:
            start=True clears the whole bank's has_written), then exp."""
            nsl = len(slices)
            qcolp = pq.tile([P, 8], F32, tag="qc")
            if ntok_pad:
                nc.vector.memset(qcolp[:], 0.0)
            for s, (c0, w) in enumerate(slices):
                for n in range(NT):
                    nc.tensor.matmul(
                        qcolp[0:w, s:s + 1],
                        lhsT=th8[:, n, c0:c0 + w],
                        rhs=wcs[:, wci, n:n + 1],
                        start=(n == 0), stop=(n == NT - 1))
            if split_exp:
                h = nsl // 2
                nc.scalar.activation(qst[:, qbase:qbase + h], qcolp[:, 0:h],
                                     AF.Exp, scale=1.0 / WSCALE)
                nc.scalar.activation(qst[:, qbase + h:qbase + nsl],
                                     qcolp[:, h:nsl],
                                     AF.Exp, scale=1.0 / WSCALE)
            else:
                nc.scalar.activation(qst[:, qbase:qbase + nsl],
                                     qcolp[:, 0:nsl],
                                     AF.Exp, scale=1.0 / WSCALE)

        def wsum(qst, qbase, nat, sb_stage, ws_tiles, bat0, tail=False):
            """fp8e3 weighted sums + inline ones-column denominator, one
            single-bank PSUM accumulator per (batch, feature-half)."""
            per_b = {}
            for s, (b_local, nat_tile, rows) in enumerate(ws_tiles):
                per_b.setdefault(b_local, []).append((s, nat_tile, rows))
            for b_local, tiles in per_b.items():
                bb = bat0 + b_local
                for off, sz in ((0, 512), (512, D + 1 - 512)):
                    ups = pu.tile([1, 512], F32, tag="ups")
                    for ci, (s, nat_tile, rows) in enumerate(tiles):
                        nc.tensor.matmul(
                            ups[:1, 0:sz],
                            lhsT=qst[0:rows, qbase + s:qbase + s + 1],
                            rhs=nat[0:rows, nat_tile, off:off + sz],
                            start=(ci == 0), stop=(ci == len(tiles) - 1))
                    if tail and off:   # ACT is drained on the tail phase
                        nc.scalar.activation(sb_stage[:1, bb, off:off + sz],
                                             ups[:1, 0:sz], AF.Copy)
                    else:
                        nc.vector.tensor_copy(sb_stage[:1, bb, off:off + sz],
                                              ups[:1, 0:sz])

        # PE stream: sc0-ph1, img-ph1 (fills sc0 tanh tail), sc0 d/ws,
        # sc1-ph1, img d/ws, sc1 d/ws (tail)
        t_slices0 = [(128 * s, P) for s in range(8)]
        i_slices = [(R * b + h * P, P if h == 0 else R - P)
                    for b in range(BPC) for h in range(2)]
        ws_img = [(b, 2 * b + h, P if h == 0 else R - P)
                  for b in range(BPC) for h in range(2)]
        th_img = ph1(w8i, xt8i, 0, ITOKP, "th", split_tanh=1)
        dexp(th_img, 1, qst_i, 0, i_slices, ntok_pad=True)
        th_sc0 = ph1(w8t, xt8t, 0, 1024, "th")
        wsum(qst_i, 0, inat, v_sb, ws_img, 0)
        nc.sync.dma_start(
            d["v_out"].rearrange("p (b n) -> p b n", b=BPC), v_sb[:, :, :])
        dexp(th_sc0, 0, qst_t, 0, t_slices0)
        th_sc1 = ph1(w8t, xt8t, 1024, 1024, "th")
        wsum(qst_t, 0, tnat, u_sb, [(s // 4, s, P) for s in range(8)], 0)
        dexp(th_sc1, 0, qst_t, 8, t_slices0, split_exp=True)
        wsum(qst_t, 8, tnat, u_sb, [(s // 4, 8 + s, P) for s in range(8)], 2,
             tail=True)
        nc.sync.dma_start(
            d["u_out"].rearrange("p (b n) -> p b n", b=BPC), u_sb[:, :, :])


def _get_nc():
    if "nc" not in _CACHE:
        _CACHE["nc"] = _build()
    return _CACHE["nc"]


def prep_core_inputs(inputs):
    """Host-side prep: slice per core, pack img tokens, cast, pre-arrange."""
    e3 = ml_dtypes.float8_e3m4
    e4 = ml_dtypes.float8_e4m3
    text = np.asarray(inputs["text_features"], dtype=np.float32)
    img = np.asarray(inputs["img_features"], dtype=np.float32)

    def warr(w):  # [D, D] -> [P, NT*KO*2*P] n-major chunks
        a = (np.asarray(w, np.float32) * WSCALE).astype(e4)
        # a[g*256 + i*128 + p, n*128 + c] -> out[p, n, g, i, c]
        return np.ascontiguousarray(
            a.reshape(KO, 2, P, NT, P).transpose(2, 3, 0, 1, 4)
        ).reshape(P, NT * KO * 2 * P)

    def wcol(w):  # wa[D:] -> [P, NT] bf16 columns (x64)
        ws = np.asarray(w, np.float32)[D:] * WSCALE
        return np.ascontiguousarray(
            ws.reshape(NT, P).T).astype(ml_dtypes.bfloat16)

    def xtarr(x):  # [T, D] -> feature-major DR layout [P, KO*2*T]
        t = x.shape[0]
        a = np.ascontiguousarray(x.T).astype(e4)   # [D, T]
        return np.ascontiguousarray(
            a.reshape(KO, 2, P, t).transpose(2, 0, 1, 3)).reshape(P, -1)

    w8t = warr(inputs["Wt2"])
    w8i = warr(inputs["Wi1"])
    wcs = np.concatenate([wcol(inputs["wa2"]), wcol(inputs["wa1"])], axis=1)

    in_maps = []
    ones_t = np.ones((TTOK, 1), np.float32)
    ones_i = np.ones((ITOKP, 1), np.float32)
    for c in range(NCORES):
        tc = text[BPC * c:BPC * (c + 1)].reshape(TTOK, D)
        ic = img[BPC * c:BPC * (c + 1)].reshape(ITOKP, D)
        inat = np.zeros((2 * BPC * P, D + 1), e3)
        ic1 = np.hstack([ic, ones_i]).astype(e3)
        for b in range(BPC):
            inat[2 * b * P:2 * b * P + R] = ic1[R * b:R * (b + 1)]
        in_maps.append({
            "w8t": w8t, "w8i": w8i, "wcs": wcs,
            "xt8t": xtarr(tc), "xt8i": xtarr(ic),
            "tnat": np.hstack([tc, ones_t]).astype(e3),
            "inat": inat,
        })
    return in_maps


def kernel(**inputs):
    nc = _get_nc()
    in_maps = prep_core_inputs(inputs)
    res = run_bass_kernel_spmd(nc, in_maps, list(range(NCORES)))
    u = np.concatenate([res.results[c]["u_out"].reshape(BPC, D + 1)
                        for c in range(NCORES)], axis=0)
    v = np.concatenate([res.results[c]["v_out"].reshape(BPC, D + 1)
                        for c in range(NCORES)], axis=0)
    u = u[:, :D] / u[:, D:D + 1]
    v = v[:, :D] / v[:, D:D + 1]
    att_text = np.broadcast_to(u[:, None, :], (B, S, D)).astype(np.float32).copy()
    att_img = np.broadcast_to(v[:, None, :], (B, S, D)).astype(np.float32).copy()
    return att_text, att_img
